# revision 26
# baseline (speedup 1.0000x reference)
"""PASA group-softmax high-pass downsample kernel for 8 Trainium2 NeuronCores.

Reference computation (n=4, c=64, h=w=128, G=2 groups, K=3, stride 2):
  xp     = reflect_pad(x, 1)
  sigma  = conv3x3(xp, conv_w)                    # [n, 18, h, w]
  sigma  = sigma * bn_scale + bn_shift            # BN (inference)
  sigma  = softmax(sigma, axis=1)                 # over all 18 channels
  sigma  = onehot(center) - sigma                 # high-pass
  out[n,g,c,i,j] = sum_k patches[n,g,c,k,i,j] * sigma[n,g,k,i,j]
  return out[:, :, ::2, ::2]                      # [4, 64, 64, 64]

Per-core layout (core = one image n, one h-half; sub-halves A/B stacked on
partitions 0-63 / 64-127 so every engine runs 128 lanes wide):

  x slab  [128, 66, 66] bf16 "phase planes": padded input rows split by
          (row, col) parity into planes p00/p01/p10/p11 at row bases
          0/17/34/50.  Every stride-2 patch view of the original conv is a
          UNIT-STRIDE view of one plane: tap (dy,dx) -> plane(dy%2, dx%2)
          rows +dy//2, cols +dx//2.  Plane p11 IS the center grid (xc).
  conv    18 matmuls (9 taps x 2 row-chunks), contraction 128 (both halves);
          tap order follows the DMA piece order so conv starts as soon as
          the first plane lands.  chunk0 -> psum parts 0:64 (36:64 zeroed
          by zero weight cols), chunk1 -> parts 64:100 (col tile 64).
  exp     per chunk on ACT (bias = BN shift) -> e bf16 (UNNORMALIZED).
  apply   per tap: ebig = esel_k @ e broadcasts E to channel layout
          (2 matmuls); prod = patch * ebig on DVE; PSUM-accumulate on PE
          via identity matmuls.  For 4 "copy taps" ACT first copies ebig
          PSUM->SBUF bf16 so the DVE multiply runs in 2x mode.
          ebig matmuls are issued 2 taps ahead of the acc matmuls so the
          DVE multiply stream never waits on the in-order PE queue.
  r-chain (off critical path, runs during the taps): D = sel @ e per chunk,
          r = 1/D (DVE fast approx), cast bf16, rbig = channel-broadcast
          matmuls, ACT copies rbig PSUM->SBUF.
  tail    t = acc * rbig (DVE), y = xc - t (DVE, f32), DMA out.
"""

import os
import ml_dtypes
import numpy as np

import concourse.bass as bass
import concourse.tile as tile
from concourse import bacc, mybir
from concourse.bass_utils import run_bass_kernel_spmd

F32 = mybir.dt.float32
BF16 = mybir.dt.bfloat16
F8 = mybir.dt.float8e4

N, C, H, W = 4, 64, 128, 128
G, K = 2, 3
K2 = K * K
EPS = 1e-5
NCORES = 8
HO, WO = H // 2, W // 2            # 64, 64 output spatial
RS = 16                            # output rows per sub-half (A/B)
CH_ROWS = RS // 2                  # 8 output rows per chunk
CHUNK = CH_ROWS * WO               # 512 positions per chunk
POS = RS * WO                      # 1024 positions per sub-half/partition

# phase-plane row bases inside the [128, 66, 66] slab
PB = {(0, 0): 0, (0, 1): 17, (1, 0): 34, (1, 1): 50}
SLAB_R, SLAB_C = 66, 66

NWARM = 6
# taps whose DVE multiply uses an ACT-copied SBUF ebig (2x mode); these
# need dx//2 == 0 so the patch view is 4-byte aligned
COPY_TAPS = (1, 3, 4, 6, 7)
# conv tap issue order grouped by the plane DMA piece that feeds it
CONV_ORDER = (0, 2, 6, 8, 4, 1, 7, 3, 5)

_compiled = None


def _tap_view(x_sb, k, rows, r0=0):
    """Unit-stride patch view for tap k: [128, rows, 64] at chunk row r0."""
    dy, dx = k // K, k % K
    base = PB[(dy % 2, dx % 2)] + dy // 2
    c0 = dx // 2
    return x_sb[:, base + r0 : base + r0 + rows, c0 : c0 + 64]


def _build_program():
    nc = bacc.Bacc(
        "TRN2", target_bir_lowering=False, debug=False, num_devices=NCORES
    )

    xab = nc.dram_tensor("xab", [128, SLAB_R, SLAB_C], BF16, kind="ExternalInput")
    wts = nc.dram_tensor("wts", [128, K2, 36], BF16, kind="ExternalInput")
    bias = nc.dram_tensor("bias", [128, 1], F32, kind="ExternalInput")
    sel = nc.dram_tensor("sel", [128, 34], BF16, kind="ExternalInput")
    rselch = nc.dram_tensor("rselch", [34, 256], BF16, kind="ExternalInput")
    esel = nc.dram_tensor("esel", [72, K2, 128], F8, kind="ExternalInput")
    ident = nc.dram_tensor("ident", [128, 128], BF16, kind="ExternalInput")
    y = nc.dram_tensor("y", [128, RS, WO], BF16, kind="ExternalOutput")
    warm_out = nc.dram_tensor("warm_out", [1, 2], F32, kind="ExternalOutput")

    with tile.TileContext(nc) as tc:
        with (
            tc.tile_pool(name="singles", bufs=1) as singles,
            tc.tile_pool(name="psum", bufs=1, space="PSUM") as psum,
            tc.tile_pool(name="ebig", bufs=2, space="PSUM") as ebig_pool,
            tc.tile_pool(name="work", bufs=3) as work,
            # deep buffers: acc/copy consumers never back-pressure the
            # DVE multiply stream
            tc.tile_pool(name="prods", bufs=9) as prod_pool,
            tc.tile_pool(name="ebsb", bufs=6) as ebsb_pool,
        ):
            # ---- constant loads on the sync ring (ident first: warm-up) ----
            ident_sb = singles.tile([128, 128], BF16)
            nc.sync.dma_start(ident_sb[:], ident.ap())
            w_sb = singles.tile([128, K2, 36], BF16)
            nc.sync.dma_start(w_sb[:], wts.ap())
            x_sb = singles.tile([128, SLAB_R, SLAB_C], BF16)
            nc.sync.dma_start(x_sb[:, 50:66], xab.ap()[:, 50:66])  # p11
            nc.sync.dma_start(x_sb[:, 42:50], xab.ap()[:, 42:50])  # p10b
            bias_sb = singles.tile([128, 1], F32)
            nc.sync.dma_start(bias_sb[:], bias.ap())
            sel_sb = singles.tile([128, 34], BF16)
            nc.sync.dma_start(sel_sb[:], sel.ap())
            rsel_sb = singles.tile([34, 256], BF16)
            nc.sync.dma_start(rsel_sb[:], rselch.ap())
            esel_sb = singles.tile([128, K2, 128], F8)
            nc.sync.dma_start(esel_sb[0:36], esel.ap()[0:36])
            nc.sync.dma_start(esel_sb[64:100], esel.ap()[36:72])

            # prewarm ACT's exp table (dep-free)
            warm_in = work.tile([1, 1], F32, tag="warm_in")
            nc.gpsimd.memset(warm_in[:], 0.25)
            warm_e = work.tile([1, 1], F32, tag="warm")
            nc.scalar.activation(warm_e[:], warm_in[:],
                                 mybir.ActivationFunctionType.Exp)

            # ---- x slab split across the gpsimd + scalar rings ----
            for eng, (r0, r1) in (
                (nc.gpsimd, (0, 9)),     # p00 chunk0 rows (conv starts here)
                (nc.scalar, (9, 17)),    # p00 chunk1 rows
                (nc.gpsimd, (17, 26)),   # p01 chunk0
                (nc.scalar, (26, 34)),   # p01 chunk1
                (nc.scalar, (34, 42)),   # p10a
            ):
                eng.dma_start(x_sb[:, r0:r1], xab.ap()[:, r0:r1])

            # short PE warm-up (clock ramp) before conv
            warm_ps = psum.tile([128, 128], F32, tag="dr",
                                padded_shape=[128, CHUNK])
            for i in range(NWARM):
                nc.tensor.matmul(warm_ps[:], ident_sb[:], ident_sb[:],
                                 start=(i == 0), stop=(i == NWARM - 1),
                                 skip_group_check=True)
            warm_sb = work.tile([1, 2], F32, tag="warm_sb")
            nc.vector.tensor_copy(warm_sb[:], warm_ps[0:1, 0:2])
            nc.sync.dma_start(warm_out.ap(), warm_sb[:])

            # ---- conv: 9 taps x 2 chunks, interleaved so the ch0 (col tile
            # 0) and ch1 (col tile 64) matmuls run concurrently; separate
            # sigma tiles per chunk keep the exp deps chunk-local ----
            sigma_ps = psum.tile([128, CHUNK], F32, tag="acc",
                               padded_shape=[128, CHUNK])
            for i, k in enumerate(CONV_ORDER):
                nc.tensor.matmul(
                    sigma_ps[0:36, :], w_sb[:, k, 0:36],
                    _tap_view(x_sb, k, CH_ROWS, r0=0),
                    start=(i == 0), stop=(i == K2 - 1),
                    tile_position=(0, 0), skip_group_check=True,
                )
                nc.tensor.matmul(
                    sigma_ps[64:100, :], w_sb[:, k, 0:36],
                    _tap_view(x_sb, k, CH_ROWS, r0=CH_ROWS),
                    start=(i == 0), stop=(i == K2 - 1),
                    tile_position=(0, 64), skip_group_check=True,
                )

            # ---- E = exp(sigma + bn_shift) bf16, one ACT op ----
            e_sb = singles.tile([128, CHUNK], BF16)
            nc.scalar.activation(
                e_sb[0:100], sigma_ps[0:100],
                mybir.ActivationFunctionType.Exp,
                bias=bias_sb[0:100], scale=1.0,
            )

            # ---- denominator tile (matmuls issued in the apply pre-loop)
            d_ps = psum.tile([64, CHUNK], F32, tag="dr",
                             padded_shape=[128, CHUNK])

            # ---- apply: acc = sum_k patch_k * E_k ----
            # chunk0 accumulates on the PE (PSUM, identity matmuls); chunk1
            # accumulates on the DVE in bf16 so the throttled PE only does
            # one acc matmul per tap
            acc_ps = psum.tile([128, CHUNK], F32, tag="acc")
            junk_ps = psum.tile([128, CHUNK], F32, tag="junkb")
            accv = singles.tile([128, CHUNK], BF16)
            ebigs = {}

            def issue_ebig(k, ch):
                if ch == 0:
                    ebigs[k] = ebig_pool.tile([128, POS], F32,
                                              name=f"ebig{k}", tag="ebig")
                nc.tensor.matmul(
                    ebigs[k][:, CHUNK * ch : CHUNK * (ch + 1)],
                    esel_sb[64 * ch : 64 * ch + 36, k, :],
                    e_sb[64 * ch : 64 * ch + 36, :],
                    tile_position=(64 * ch, 0),
                    skip_group_check=True,
                )

            def issue_junk(n):
                for _ in range(n):
                    nc.tensor.matmul(junk_ps[:], ident_sb[:],
                                     x_sb[:, 0:CH_ROWS, 0:WO],
                                     start=True, stop=True,
                                     skip_group_check=True)

            r_sb = singles.tile([34, CHUNK], F32)
            r_bf = singles.tile([34, CHUNK], BF16)
            rbig0 = psum.tile([128, CHUNK], F32, tag="dr")
            rbig1 = psum.tile([128, CHUNK], F32, tag="dr2")
            rbig_sb = singles.tile([128, 2, CHUNK], BF16)

            # pre-loop: chunk-wise so the ch0 work never waits on exp1
            issue_junk(2)
            nc.tensor.matmul(d_ps[0:32, :], sel_sb[0:36, 0:32], e_sb[0:36, :],
                             tile_position=(0, 0), skip_group_check=True)
            issue_ebig(0, 0)
            issue_ebig(1, 0)
            nc.tensor.matmul(d_ps[32:34, :], sel_sb[64:100, 32:34],
                             e_sb[64:100, :],
                             tile_position=(64, 32), skip_group_check=True)
            issue_ebig(0, 1)
            issue_ebig(1, 1)
            issue_junk(1)

            prods = []
            for k in range(K2):
                eb = ebigs[k]
                if k in COPY_TAPS:
                    # ACT copies PSUM->SBUF bf16 so the multiply runs 2x
                    eb_sb = ebsb_pool.tile([128, POS], BF16, name=f"ebsb{k}",
                                           tag="ebsb")
                    nc.scalar.copy(eb_sb[:], eb[:])
                    eb = eb_sb
                prod = prod_pool.tile([128, POS], BF16, name=f"prod{k}",
                                      tag="prod")
                if k == 0:
                    # split per chunk: the ch0 half starts before exp1/ebig
                    # ch1 are even done
                    for ch in range(2):
                        cc = slice(CHUNK * ch, CHUNK * (ch + 1))
                        nc.vector.tensor_mul(
                            prod[:, cc].rearrange("p (r c) -> p r c",
                                                  r=CH_ROWS),
                            _tap_view(x_sb, k, CH_ROWS, r0=CH_ROWS * ch),
                            eb[:, cc].rearrange("p (r c) -> p r c",
                                                r=CH_ROWS),
                        )
                else:
                    nc.vector.tensor_mul(
                        prod[:].rearrange("p (r c) -> p r c", r=RS),
                        _tap_view(x_sb, k, RS),
                        eb[:].rearrange("p (r c) -> p r c", r=RS),
                    )
                prods.append(prod)
                # chunk1 accumulate on DVE (bf16)
                if k == 0:
                    nc.vector.tensor_copy(accv[:], prod[:, CHUNK:POS])
                else:
                    nc.vector.tensor_add(accv[:], accv[:],
                                         prod[:, CHUNK:POS])
                if k == 1:
                    # r-chain DVE ops parked behind the second multiply
                    nc.vector.reciprocal_approx_fast(r_sb[:], d_ps[0:34, :])
                    nc.vector.tensor_copy(r_bf[:], r_sb[:])
                # chunk0 accumulate on PE
                if k >= 1:
                    nc.tensor.matmul(
                        acc_ps[:], ident_sb[:], prods[k - 1][:, 0:CHUNK],
                        start=(k == 1), stop=False, skip_group_check=True,
                    )
                if k + 2 < K2:
                    issue_ebig(k + 2, 0)
                    issue_ebig(k + 2, 1)
                if k == 1:
                    # channel-layout broadcast of r (needed only at the tail)
                    nc.tensor.matmul(rbig0[:], rsel_sb[:, 0:128], r_bf[:])
                    nc.tensor.matmul(rbig1[:], rsel_sb[:, 128:256], r_bf[:])
                    nc.scalar.copy(rbig_sb[:, 0], rbig0[:])
                    nc.scalar.copy(rbig_sb[:, 1], rbig1[:])
            nc.tensor.matmul(acc_ps[:], ident_sb[:], prods[K2 - 1][:, 0:CHUNK],
                             start=False, stop=True, skip_group_check=True)

            # ---- tail: t = acc * rbig ; y = xc - t ; store ----
            t_sb = work.tile([128, POS], BF16, tag="tsb")
            y_sb = work.tile([128, RS, WO], BF16)
            for ch in range(2):
                rr = slice(CH_ROWS * ch, CH_ROWS * (ch + 1))
                cc = slice(CHUNK * ch, CHUNK * (ch + 1))
                src_acc = acc_ps[:] if ch == 0 else accv[:]
                nc.vector.tensor_mul(t_sb[:, cc], src_acc, rbig_sb[:, ch])
                nc.vector.tensor_sub(
                    y_sb[:, rr],
                    _tap_view(x_sb, 4, CH_ROWS, r0=CH_ROWS * ch),
                    t_sb[:, cc].rearrange("p (r c) -> p r c", r=CH_ROWS),
                )
                eng = nc.sync if ch == 0 else nc.gpsimd
                eng.dma_start(y.ap()[:, rr], y_sb[:, rr])

    nc.compile()
    return nc


def _host_inputs(x, conv_w, gamma, beta, running_mean, running_var):
    """Prepare per-core input dicts (sharding + BN folding + phase planes)."""
    scale = gamma / np.sqrt(running_var + EPS)
    shift = beta - running_mean * scale

    # conv weights: lhsT [128, 9, 64]; block-diag over sub-halves
    w_scaled = conv_w * scale[:, None, None, None]            # [18, 64, 3, 3]
    wts = np.zeros((128, K2, 36), np.float32)
    for k in range(K2):
        dy, dx = k // K, k % K
        wl = w_scaled[:, :, dy, dx].T                         # [64c, 18o]
        wts[0:64, k, 0:18] = wl
        wts[64:128, k, 18:36] = wl
    wts = wts.astype(ml_dtypes.bfloat16)

    # exp bias rows: compact layout p = 18*half + (g*9+k)
    bias = np.zeros((128, 1), np.float32)
    bias[0:18, 0] = shift
    bias[18:36, 0] = shift
    bias[64:82, 0] = shift
    bias[82:100, 0] = shift

    # D selector: ch0 -> d rows 0/1 (cols 2:32 dup col0 keeps recip finite),
    # ch1 -> d rows 32/33
    sel = np.zeros((128, 34), np.float32)
    sel[0:18, 0] = 1.0
    sel[18:36, 1] = 1.0
    sel[0:18, 2:32] = 1.0
    sel[64:82, 32] = 1.0
    sel[82:100, 33] = 1.0
    sel = sel.astype(ml_dtypes.bfloat16)

    # channel-layout r broadcast: d row -> channel partitions, per chunk
    rselch = np.zeros((34, 256), np.float32)
    rselch[0, 0:64] = 1.0      # ch0, A half
    rselch[1, 64:128] = 1.0    # ch0, B half
    rselch[32, 128:192] = 1.0  # ch1, A half
    rselch[33, 192:256] = 1.0  # ch1, B half
    rselch = rselch.astype(ml_dtypes.bfloat16)

    # tap selector: rows 0:36 for ch0, rows 36:72 load at partitions 64:100
    # channel partition j: f-row 18*(j//64) + 9*((j%64)//32) + k
    esel = np.zeros((72, K2, 128), np.float32)
    for k in range(K2):
        for j in range(128):
            r = 18 * (j // 64) + 9 * ((j % 64) // 32) + k
            esel[r, k, j] = 1.0
            esel[36 + r, k, j] = 1.0
    esel = esel.astype(ml_dtypes.float8_e4m3fn)

    ident = np.eye(128, dtype=np.float32).astype(ml_dtypes.bfloat16)

    xpad = np.pad(x, ((0, 0), (0, 0), (1, 1), (1, 1)), mode="reflect")

    in_maps = []
    for core in range(NCORES):
        n, h = core // 2, core % 2
        r0 = 64 * h
        xa = np.zeros((128, SLAB_R, SLAB_C), np.float32)
        for half, rs in ((0, r0), (1, r0 + 32)):
            slab = xpad[n, :, rs : rs + 33, :]                # [64, 33, 130]
            p = slice(64 * half, 64 * half + 64)
            xa[p, 0:17, 0:65] = slab[:, 0::2, 0::2]
            xa[p, 17:34, 0:65] = slab[:, 0::2, 1::2]
            xa[p, 34:50, 0:65] = slab[:, 1::2, 0::2]
            xa[p, 50:66, 0:65] = slab[:, 1::2, 1::2]
        in_maps.append(
            {"xab": xa.astype(ml_dtypes.bfloat16), "wts": wts, "bias": bias,
             "sel": sel, "rselch": rselch, "esel": esel, "ident": ident}
        )
    return in_maps


def _gather_output(results):
    out = np.empty((N, C, HO, WO), np.float32)
    for core, res in enumerate(results):
        n, h = core // 2, core % 2
        ycore = res["y"].reshape(2, C, RS, WO)
        out[n, :, 32 * h : 32 * h + RS, :] = ycore[0]
        out[n, :, 32 * h + RS : 32 * h + 2 * RS, :] = ycore[1]
    return out


def _ensure_ntff_hook():
    """Install the axon NTFF profile hook if the image's antenv lacks it."""
    try:
        from antenv import axon_hooks  # noqa: F401
        return
    except ImportError:
        pass
    try:
        import sys
        import types

        import antenv
        from trn_agent_boot.trn_boot import _ntff_profile_via_ctypes

        hook = _ntff_profile_via_ctypes("/opt/axon/libaxon_pjrt.so")
        mod = types.ModuleType("antenv.axon_hooks")
        state = {"hook": hook}
        mod.get_axon_ntff_profile_hook = lambda: state["hook"]
        mod.set_axon_ntff_profile_hook = lambda h: state.update(hook=h)
        sys.modules["antenv.axon_hooks"] = mod
        antenv.axon_hooks = mod
    except Exception:
        pass


def kernel(x, conv_w, gamma, beta, running_mean, running_var):
    global _compiled
    x = np.asarray(x, np.float32)
    conv_w = np.asarray(conv_w, np.float32)
    gamma = np.asarray(gamma, np.float32)
    beta = np.asarray(beta, np.float32)
    running_mean = np.asarray(running_mean, np.float32)
    running_var = np.asarray(running_var, np.float32)

    if _compiled is None:
        _compiled = _build_program()
    nc = _compiled

    in_maps = _host_inputs(x, conv_w, gamma, beta, running_mean, running_var)
    trace = bool(int(os.environ.get("PASA_TRACE", "0")))
    if trace:
        _ensure_ntff_hook()
    res = run_bass_kernel_spmd(
        nc, in_maps, core_ids=list(range(NCORES)), trace=trace
    )
    kernel.last_results = res
    return _gather_output(res.results)


if __name__ == "__main__":
    # quick CoreSim check of core 0 against a numpy re-implementation
    from concourse.bass_interp import CoreSim

    rng = np.random.default_rng(0)
    x = rng.standard_normal((N, C, H, W), np.float32)
    conv_w = (rng.standard_normal((G * K2, C, K, K), np.float32)
              * np.sqrt(2.0 / (G * K2 * K * K)))
    gamma = rng.uniform(0.5, 1.5, G * K2).astype(np.float32)
    beta = (rng.standard_normal(G * K2) * 0.1).astype(np.float32)
    rmean = (rng.standard_normal(G * K2) * 0.1).astype(np.float32)
    rvar = rng.uniform(0.5, 1.5, G * K2).astype(np.float32)

    nc = _build_program()
    in_maps = _host_inputs(x, conv_w, gamma, beta, rmean, rvar)
    sim = CoreSim(nc)
    for k, v in in_maps[0].items():
        sim.tensor(k)[:] = v
    sim.simulate(check_with_hw=False)
    ysim = np.array(sim.tensor("y")).reshape(2, C, RS, WO)

    # numpy reference for core 0 region (image 0, output rows 0..32)
    scale = gamma / np.sqrt(rvar + EPS)
    shift = beta - rmean * scale
    xpad = np.pad(x[0], ((0, 0), (1, 1), (1, 1)), mode="reflect")
    sig = np.zeros((G * K2, 32, WO), np.float32)
    for o in range(G * K2):
        for dy in range(K):
            for dx in range(K):
                sig[o] += np.einsum(
                    "crw->rw",
                    conv_w[o, :, dy, dx][:, None, None]
                    * xpad[:, dy : dy + 64 : 2, dx : dx + 128 : 2],
                )
    sig = sig * scale[:, None, None] + shift[:, None, None]
    e = np.exp(sig)
    r = 1.0 / e.sum(0)
    acc = np.zeros((C, 32, WO), np.float32)
    for g in range(G):
        for k in range(K2):
            dy, dx = k // K, k % K
            acc[32 * g : 32 * g + 32] += (
                xpad[32 * g : 32 * g + 32, dy : dy + 64 : 2, dx : dx + 128 : 2]
                * e[g * K2 + k][None]
            )
    ref = (xpad[:, 1:65:2, 1:129:2] - acc * r[None]).astype(np.float32)

    got = np.concatenate([ysim[0], ysim[1]], axis=1)
    err = np.abs(got - ref).max() / np.abs(ref).max()
    print("sim rel err:", err)
